# revision 6
# baseline (speedup 1.0000x reference)
"""BitLinear (ternary-quantized linear) Trainium2 kernel.

out = x @ (gamma * ternary(weight)).T + bias, computed tensor-parallel over
8 NeuronCores: weight/bias sharded along out_features, x replicated.

The device program is a pure bf16 matmul streamer: all input preparation
(gamma, ternary quantization of the weight shard, bf16 cast of x, and layout
tiling so every DMA line is long and contiguous) happens on host, where it is
exact fp32 math identical to the reference's. Per core:

  1. DMA the pre-tiled quantized weight shard (4 x 4 MiB, n-block-major) and
     the first x tiles into SBUF.
  2. 8192 bf16 128x128x512 matmuls accumulating fp32 in PSUM: for each of 64
     m-tiles, kt-outer/nb-inner so one LDWEIGHTS (stationary x tile) serves 4
     matmuls and the PE never idles. m-tile 0 runs nb-outer so matmuls start
     as soon as the first weight chunk lands.
  3. Drain: psum * gamma + bias on DVE, DMA out on the Scalar queue.

gamma = max(mean(|clip(w, -2, 2)|), 1e-4) is computed on host with the same
jnp ops the module uses so the quantization boundary matches bit-exactly.
"""

import numpy as np
import ml_dtypes

import concourse.bass as bass
import concourse.mybir as mybir
import concourse.tile as tile
from concourse import bacc
from concourse.bass_utils import run_bass_kernel_spmd

P = 128
B, S, D_IN, D_OUT = 4, 2048, 4096, 16384
M = B * S                 # 8192 tokens
K = D_IN                  # 4096 contraction
N_CORES = 8
NS = D_OUT // N_CORES     # 2048 out-features per core
KT = K // P               # 32 k-subtiles
MT = M // P               # 64 m-tiles
NBS = 512                 # psum bank free size (fp32)
NB = NS // NBS            # 4 psum n-blocks

F32 = mybir.dt.float32
BF16 = mybir.dt.bfloat16

_NC_CACHE = None
LAST_RESULTS = None


def _build_nc():
    nc = bacc.Bacc(None, target_bir_lowering=False, debug=False)

    # host-tiled inputs: xt[j][p][kt*128+m] = x[j*128+m, kt*128+p] (bf16)
    #                    wq[nb][p][kt*512+n] = ternary_w[nb*512+n, kt*128+p]
    xt_in = nc.declare_dram_parameter("xt", [MT, P, K], BF16, isOutput=False)
    wq_in = nc.declare_dram_parameter("wq", [NB, P, KT * NBS], BF16, isOutput=False)
    b_in = nc.declare_dram_parameter("bias", [P, NS], F32, isOutput=False)
    s_in = nc.declare_dram_parameter("scal", [P, 1], F32, isOutput=False)
    y_out = nc.declare_dram_parameter("out", [M, NS], F32, isOutput=True)

    with tile.TileContext(nc) as tc:
        with (
            tc.tile_pool(name="const", bufs=1) as constp,
            tc.tile_pool(name="xt", bufs=4) as xtp,
            tc.tile_pool(name="osb", bufs=3) as osbp,
            tc.tile_pool(name="psum", bufs=8, space="PSUM") as psump,
        ):
            # x tile 0 is the first matmul's stationary operand: it goes
            # first on the Sync queue (emitted below, j=0 iteration)
            wq_sb = constp.tile([P, NB, KT * NBS], BF16)
            scal = constp.tile([P, 1], F32)
            bias_sb = constp.tile([P, NS], F32)

            for j in range(MT):
                xt_t = xtp.tile([P, K], BF16, tag="xt", name=f"xt_{j}")
                nc.sync.dma_start(out=xt_t[:], in_=xt_in[j])
                if j == 0:
                    # weight chunks split across both HWDGE queues right
                    # behind x tile 0, halving wq0 so the first matmul's gate
                    # (wq chunk 0) lands in ~6us; scal/bias ride the idle
                    # SWDGE queue (first needed by the m-tile-0 drain)
                    H = P // 2
                    nc.sync.dma_start(out=wq_sb[:H, 0, :], in_=wq_in[0][:H])
                    nc.scalar.dma_start(out=wq_sb[H:, 0, :], in_=wq_in[0][H:])
                    nc.sync.dma_start(out=wq_sb[:, 1, :], in_=wq_in[1])
                    nc.scalar.dma_start(out=wq_sb[:, 2, :], in_=wq_in[2])
                    nc.scalar.dma_start(out=wq_sb[:, 3, :], in_=wq_in[3])
                    nc.gpsimd.dma_start(out=scal[:], in_=s_in[:])
                    nc.gpsimd.dma_start(out=bias_sb[:], in_=b_in[:])
                psums = [
                    psump.tile([P, NBS], F32, tag="ps", name=f"ps_{j}_{nb}")
                    for nb in range(NB)
                ]
                if j == 0:
                    # nb-outer: matmuls gate on one weight chunk at a time,
                    # so the PE starts as soon as wq chunk 0 lands
                    for nb in range(NB):
                        for kt in range(KT):
                            nc.tensor.matmul(
                                psums[nb][:],
                                xt_t[:, kt * P:(kt + 1) * P],
                                wq_sb[:, nb, kt * NBS:(kt + 1) * NBS],
                                start=(kt == 0),
                                stop=(kt == KT - 1),
                            )
                else:
                    # kt-outer/nb-inner: one stationary load per kt serves 4
                    # matmuls; LDWEIGHTS for kt+1 is pulled ahead by the PE's
                    # reorder window and hides behind kt's matmuls
                    for kt in range(KT):
                        for nb in range(NB):
                            nc.tensor.matmul(
                                psums[nb][:],
                                xt_t[:, kt * P:(kt + 1) * P],
                                wq_sb[:, nb, kt * NBS:(kt + 1) * NBS],
                                start=(kt == 0),
                                stop=(kt == KT - 1),
                            )
                osb = osbp.tile([P, NS], F32, tag="osb", name=f"osb_{j}")
                if j == MT - 1:
                    # pipeline the final drain per n-block so the last store
                    # isn't gated on the full 2048-wide scale+bias pass
                    for nb in range(NB):
                        sl = slice(nb * NBS, (nb + 1) * NBS)
                        nc.vector.tensor_scalar(
                            osb[:, sl], psums[nb][:], scal[:, 0:1], None,
                            mybir.AluOpType.mult,
                        )
                        nc.vector.tensor_tensor(
                            osb[:, sl], osb[:, sl], bias_sb[:, sl],
                            mybir.AluOpType.add,
                        )
                        nc.scalar.dma_start(
                            out=y_out[j * P:(j + 1) * P, sl], in_=osb[:, sl]
                        )
                else:
                    for nb in range(NB):
                        nc.vector.tensor_scalar(
                            osb[:, nb * NBS:(nb + 1) * NBS],
                            psums[nb][:],
                            scal[:, 0:1],
                            None,
                            mybir.AluOpType.mult,
                        )
                    nc.vector.tensor_tensor(
                        osb[:], osb[:], bias_sb[:], mybir.AluOpType.add
                    )
                    nc.scalar.dma_start(out=y_out[j * P:(j + 1) * P, :], in_=osb[:])

    nc.compile()
    return nc


def _compute_gamma(weight: np.ndarray) -> np.float32:
    """Replicate the module's gamma computation bit-exactly (jnp, fp32)."""
    import jax
    import jax.numpy as jnp

    with jax.default_device(jax.devices("cpu")[0]):
        w_f32 = jnp.clip(jnp.asarray(weight, dtype=jnp.float32), -2.0, 2.0)
        gamma = jnp.maximum(jnp.mean(jnp.abs(w_f32)), 1e-4)
        return np.float32(np.asarray(gamma))


def kernel(x: np.ndarray, weight: np.ndarray, bias: np.ndarray) -> np.ndarray:
    global _NC_CACHE, LAST_RESULTS

    x2d = np.asarray(x, dtype=np.float32).reshape(M, K)
    weight = np.asarray(weight, dtype=np.float32)
    bias = np.asarray(bias, dtype=np.float32)

    gamma = _compute_gamma(weight)
    scal = np.full((P, 1), gamma, dtype=np.float32)

    # x: bf16 cast (same RNE rounding the reference path would see from a
    # device-side cast), tiled to [MT, P(k_sub), kt*128+m]
    xt = np.ascontiguousarray(
        x2d.astype(ml_dtypes.bfloat16)
        .reshape(MT, P, KT, P)
        .transpose(0, 3, 2, 1)
        .reshape(MT, P, K)
    )

    # ternary quantization, exact fp32 math as in the reference
    w_f32 = np.clip(weight, -2.0, 2.0)
    w_t = np.clip(np.round(w_f32 / gamma), -1.0, 1.0).astype(ml_dtypes.bfloat16)

    if _NC_CACHE is None:
        _NC_CACHE = _build_nc()
    nc = _NC_CACHE

    in_maps = []
    for i in range(N_CORES):
        wq_shard = np.ascontiguousarray(
            w_t[i * NS:(i + 1) * NS]              # [2048, 4096] ternary bf16
            .reshape(NB, NBS, KT, P)
            .transpose(0, 3, 2, 1)                # [nb, p, kt, n]
            .reshape(NB, P, KT * NBS)
        )
        b_shard = np.ascontiguousarray(
            np.broadcast_to(bias[i * NS:(i + 1) * NS], (P, NS))
        )
        in_maps.append({"xt": xt, "wq": wq_shard, "bias": b_shard, "scal": scal})

    res = run_bass_kernel_spmd(nc, in_maps, list(range(N_CORES)))
    LAST_RESULTS = res

    out = np.concatenate([res.results[i]["out"] for i in range(N_CORES)], axis=1)
    return np.ascontiguousarray(out.reshape(B, S, D_OUT))


# revision 7
# speedup vs baseline: 1.2029x; 1.2029x over previous
"""BitLinear (ternary-quantized linear) Trainium2 kernel.

out = x @ (gamma * ternary(weight)).T + bias, computed tensor-parallel over
8 NeuronCores: weight/bias sharded along out_features, x replicated.

The device program is a pure bf16 matmul streamer: all input preparation
(gamma, ternary quantization of the weight shard, bf16 cast of x, and layout
tiling so every DMA line is long and contiguous) happens on host, where it is
exact fp32 math identical to the reference's. Per core:

  1. DMA the pre-tiled quantized weight shard (4 x 4 MiB, n-block-major) and
     the first x tiles into SBUF.
  2. 8192 bf16 128x128x512 matmuls accumulating fp32 in PSUM: for each of 64
     m-tiles, kt-outer/nb-inner so one LDWEIGHTS (stationary x tile) serves 4
     matmuls and the PE never idles. m-tile 0 runs nb-outer so matmuls start
     as soon as the first weight chunk lands.
  3. Drain: psum * gamma + bias on DVE, DMA out on the Scalar queue.

gamma = max(mean(|clip(w, -2, 2)|), 1e-4) is computed on host with the same
jnp ops the module uses so the quantization boundary matches bit-exactly.
"""

import numpy as np
import ml_dtypes

import concourse.bass as bass
import concourse.mybir as mybir
import concourse.tile as tile
from concourse import bacc
from concourse.bass_utils import run_bass_kernel_spmd

P = 128
B, S, D_IN, D_OUT = 4, 2048, 4096, 16384
M = B * S                 # 8192 tokens
K = D_IN                  # 4096 contraction
N_CORES = 8
NS = D_OUT // N_CORES     # 2048 out-features per core
KT = K // P               # 32 k-subtiles
MT = M // P               # 64 m-tiles
NBS = 512                 # psum bank free size (fp32)
NB = NS // NBS            # 4 psum n-blocks

F32 = mybir.dt.float32
BF16 = mybir.dt.bfloat16

_NC_CACHE = None
LAST_RESULTS = None


def _build_nc():
    nc = bacc.Bacc(None, target_bir_lowering=False, debug=False)

    # host-tiled inputs: xt[j][p][kt*128+m] = x[j*128+m, kt*128+p] (bf16)
    #                    wq[nb][p][kt*512+n] = ternary_w[nb*512+n, kt*128+p]
    xt_in = nc.declare_dram_parameter("xt", [MT, P, K], BF16, isOutput=False)
    wq_in = nc.declare_dram_parameter("wq", [NB, P, KT * NBS], BF16, isOutput=False)
    b_in = nc.declare_dram_parameter("bias", [P, NS], F32, isOutput=False)
    s_in = nc.declare_dram_parameter("scal", [P, 1], F32, isOutput=False)
    y_out = nc.declare_dram_parameter("out", [M, NS], F32, isOutput=True)

    with tile.TileContext(nc) as tc:
        with (
            tc.tile_pool(name="const", bufs=1) as constp,
            tc.tile_pool(name="xt", bufs=4) as xtp,
            tc.tile_pool(name="osb", bufs=3) as osbp,
            tc.tile_pool(name="psum", bufs=8, space="PSUM") as psump,
        ):
            # x tile 0 is the first matmul's stationary operand: it goes
            # first on the Sync queue (emitted below, j=0 iteration)
            wq_sb = constp.tile([P, NB, KT * NBS], BF16)
            scal = constp.tile([P, 1], F32)
            bias_sb = constp.tile([P, NS], F32)

            for j in range(MT):
                xt_t = xtp.tile([P, K], BF16, tag="xt", name=f"xt_{j}")
                nc.sync.dma_start(out=xt_t[:], in_=xt_in[j])
                if j == 0:
                    # weight chunks split across both HWDGE queues right
                    # behind x tile 0, halving wq0 so the first matmul's gate
                    # (wq chunk 0) lands in ~6us; scal/bias ride the idle
                    # SWDGE queue (first needed by the m-tile-0 drain)
                    # (SWDGE/gpsimd is avoided here: its software descriptor
                    # startup stalls the DMA path ~30us at kernel start)
                    H = P // 2
                    nc.sync.dma_start(out=wq_sb[:H, 0, :], in_=wq_in[0][:H])
                    nc.scalar.dma_start(out=wq_sb[H:, 0, :], in_=wq_in[0][H:])
                    nc.sync.dma_start(out=wq_sb[:, 1, :], in_=wq_in[1])
                    nc.scalar.dma_start(out=wq_sb[:, 2, :], in_=wq_in[2])
                    nc.sync.dma_start(out=scal[:], in_=s_in[:])
                    nc.scalar.dma_start(out=wq_sb[:, 3, :], in_=wq_in[3])
                    nc.scalar.dma_start(out=bias_sb[:], in_=b_in[:])
                psums = [
                    psump.tile([P, NBS], F32, tag="ps", name=f"ps_{j}_{nb}")
                    for nb in range(NB)
                ]
                if j == 0:
                    # nb-outer: matmuls gate on one weight chunk at a time,
                    # so the PE starts as soon as wq chunk 0 lands
                    for nb in range(NB):
                        for kt in range(KT):
                            nc.tensor.matmul(
                                psums[nb][:],
                                xt_t[:, kt * P:(kt + 1) * P],
                                wq_sb[:, nb, kt * NBS:(kt + 1) * NBS],
                                start=(kt == 0),
                                stop=(kt == KT - 1),
                            )
                else:
                    # kt-outer/nb-inner: one stationary load per kt serves 4
                    # matmuls; LDWEIGHTS for kt+1 is pulled ahead by the PE's
                    # reorder window and hides behind kt's matmuls
                    for kt in range(KT):
                        for nb in range(NB):
                            nc.tensor.matmul(
                                psums[nb][:],
                                xt_t[:, kt * P:(kt + 1) * P],
                                wq_sb[:, nb, kt * NBS:(kt + 1) * NBS],
                                start=(kt == 0),
                                stop=(kt == KT - 1),
                            )
                osb = osbp.tile([P, NS], F32, tag="osb", name=f"osb_{j}")
                if j == MT - 1:
                    # pipeline the final drain per n-block so the last store
                    # isn't gated on the full 2048-wide scale+bias pass
                    for nb in range(NB):
                        sl = slice(nb * NBS, (nb + 1) * NBS)
                        nc.vector.tensor_scalar(
                            osb[:, sl], psums[nb][:], scal[:, 0:1], None,
                            mybir.AluOpType.mult,
                        )
                        nc.vector.tensor_tensor(
                            osb[:, sl], osb[:, sl], bias_sb[:, sl],
                            mybir.AluOpType.add,
                        )
                        nc.scalar.dma_start(
                            out=y_out[j * P:(j + 1) * P, sl], in_=osb[:, sl]
                        )
                else:
                    for nb in range(NB):
                        nc.vector.tensor_scalar(
                            osb[:, nb * NBS:(nb + 1) * NBS],
                            psums[nb][:],
                            scal[:, 0:1],
                            None,
                            mybir.AluOpType.mult,
                        )
                    nc.vector.tensor_tensor(
                        osb[:], osb[:], bias_sb[:], mybir.AluOpType.add
                    )
                    nc.scalar.dma_start(out=y_out[j * P:(j + 1) * P, :], in_=osb[:])

    nc.compile()
    return nc


def _compute_gamma(weight: np.ndarray) -> np.float32:
    """Replicate the module's gamma computation bit-exactly (jnp, fp32)."""
    import jax
    import jax.numpy as jnp

    with jax.default_device(jax.devices("cpu")[0]):
        w_f32 = jnp.clip(jnp.asarray(weight, dtype=jnp.float32), -2.0, 2.0)
        gamma = jnp.maximum(jnp.mean(jnp.abs(w_f32)), 1e-4)
        return np.float32(np.asarray(gamma))


def kernel(x: np.ndarray, weight: np.ndarray, bias: np.ndarray) -> np.ndarray:
    global _NC_CACHE, LAST_RESULTS

    x2d = np.asarray(x, dtype=np.float32).reshape(M, K)
    weight = np.asarray(weight, dtype=np.float32)
    bias = np.asarray(bias, dtype=np.float32)

    gamma = _compute_gamma(weight)
    scal = np.full((P, 1), gamma, dtype=np.float32)

    # x: bf16 cast (same RNE rounding the reference path would see from a
    # device-side cast), tiled to [MT, P(k_sub), kt*128+m]
    xt = np.ascontiguousarray(
        x2d.astype(ml_dtypes.bfloat16)
        .reshape(MT, P, KT, P)
        .transpose(0, 3, 2, 1)
        .reshape(MT, P, K)
    )

    # ternary quantization, exact fp32 math as in the reference
    w_f32 = np.clip(weight, -2.0, 2.0)
    w_t = np.clip(np.round(w_f32 / gamma), -1.0, 1.0).astype(ml_dtypes.bfloat16)

    if _NC_CACHE is None:
        _NC_CACHE = _build_nc()
    nc = _NC_CACHE

    in_maps = []
    for i in range(N_CORES):
        wq_shard = np.ascontiguousarray(
            w_t[i * NS:(i + 1) * NS]              # [2048, 4096] ternary bf16
            .reshape(NB, NBS, KT, P)
            .transpose(0, 3, 2, 1)                # [nb, p, kt, n]
            .reshape(NB, P, KT * NBS)
        )
        b_shard = np.ascontiguousarray(
            np.broadcast_to(bias[i * NS:(i + 1) * NS], (P, NS))
        )
        in_maps.append({"xt": xt, "wq": wq_shard, "bias": b_shard, "scal": scal})

    res = run_bass_kernel_spmd(nc, in_maps, list(range(N_CORES)))
    LAST_RESULTS = res

    out = np.concatenate([res.results[i]["out"] for i in range(N_CORES)], axis=1)
    return np.ascontiguousarray(out.reshape(B, S, D_OUT))


# revision 8
# speedup vs baseline: 1.2080x; 1.0042x over previous
"""BitLinear (ternary-quantized linear) Trainium2 kernel.

out = x @ (gamma * ternary(weight)).T + bias, computed tensor-parallel over
8 NeuronCores: weight/bias sharded along out_features, x replicated.

The device program is a pure bf16 matmul streamer: all input preparation
(gamma, ternary quantization of the weight shard, bf16 cast of x, and layout
tiling so every DMA line is long and contiguous) happens on host, where it is
exact fp32 math identical to the reference's. Per core:

  1. DMA the pre-tiled quantized weight shard (4 x 4 MiB, n-block-major) and
     the first x tiles into SBUF.
  2. 8192 bf16 128x128x512 matmuls accumulating fp32 in PSUM: for each of 64
     m-tiles, kt-outer/nb-inner so one LDWEIGHTS (stationary x tile) serves 4
     matmuls and the PE never idles. m-tile 0 runs nb-outer so matmuls start
     as soon as the first weight chunk lands.
  3. Drain: psum * gamma + bias on DVE, DMA out on the Scalar queue.

gamma = max(mean(|clip(w, -2, 2)|), 1e-4) is computed on host with the same
jnp ops the module uses so the quantization boundary matches bit-exactly.
"""

import numpy as np
import ml_dtypes

import concourse.bass as bass
import concourse.mybir as mybir
import concourse.tile as tile
from concourse import bacc
from concourse.bass_utils import run_bass_kernel_spmd

P = 128
B, S, D_IN, D_OUT = 4, 2048, 4096, 16384
M = B * S                 # 8192 tokens
K = D_IN                  # 4096 contraction
N_CORES = 8
NS = D_OUT // N_CORES     # 2048 out-features per core
KT = K // P               # 32 k-subtiles
MT = M // P               # 64 m-tiles
NBS = 512                 # psum bank free size (fp32)
NB = NS // NBS            # 4 psum n-blocks

F32 = mybir.dt.float32
BF16 = mybir.dt.bfloat16

_NC_CACHE = None
LAST_RESULTS = None


def _build_nc():
    nc = bacc.Bacc(None, target_bir_lowering=False, debug=False)

    # host-tiled inputs: xt[j][p][kt*128+m] = x[j*128+m, kt*128+p] (bf16)
    #                    wq[nb][p][kt*512+n] = ternary_w[nb*512+n, kt*128+p]
    xt_in = nc.declare_dram_parameter("xt", [MT, P, K], BF16, isOutput=False)
    wq_in = nc.declare_dram_parameter("wq", [NB, P, KT * NBS], BF16, isOutput=False)
    b_in = nc.declare_dram_parameter("bias", [P, NS], F32, isOutput=False)
    s_in = nc.declare_dram_parameter("scal", [P, 1], F32, isOutput=False)
    y_out = nc.declare_dram_parameter("out", [M, NS], F32, isOutput=True)

    with tile.TileContext(nc) as tc:
        with (
            tc.tile_pool(name="const", bufs=1) as constp,
            tc.tile_pool(name="xt", bufs=4) as xtp,
            tc.tile_pool(name="osb", bufs=3) as osbp,
            tc.tile_pool(name="psum", bufs=8, space="PSUM") as psump,
        ):
            # x tile 0 is the first matmul's stationary operand: it goes
            # first on the Sync queue (emitted below, j=0 iteration)
            wq_sb = constp.tile([P, NB, KT * NBS], BF16)
            scal = constp.tile([P, 1], F32)
            bias_sb = constp.tile([P, NS], F32)

            for j in range(MT):
                xt_t = xtp.tile([P, K], BF16, tag="xt", name=f"xt_{j}")
                nc.sync.dma_start(out=xt_t[:], in_=xt_in[j])
                if j == 0:
                    # weight chunks + bias all on the Scalar queue (its only
                    # other use, output stores, starts much later); x tiles +
                    # scal on Sync. Each HWDGE queue has ~20us fixed startup
                    # to first completion, so the first-matmul gate is one
                    # 1 MiB xt DMA + one 4 MiB wq chunk, in parallel.
                    # (SWDGE/gpsimd is avoided: its software descriptor
                    # startup stalls the DMA path ~30us at kernel start.)
                    nc.sync.dma_start(out=scal[:], in_=s_in[:])
                    for nb in range(NB):
                        nc.scalar.dma_start(out=wq_sb[:, nb, :], in_=wq_in[nb])
                    nc.scalar.dma_start(out=bias_sb[:], in_=b_in[:])
                psums = [
                    psump.tile([P, NBS], F32, tag="ps", name=f"ps_{j}_{nb}")
                    for nb in range(NB)
                ]
                if j == 0:
                    # nb-outer: matmuls gate on one weight chunk at a time,
                    # so the PE starts as soon as wq chunk 0 lands
                    for nb in range(NB):
                        for kt in range(KT):
                            nc.tensor.matmul(
                                psums[nb][:],
                                xt_t[:, kt * P:(kt + 1) * P],
                                wq_sb[:, nb, kt * NBS:(kt + 1) * NBS],
                                start=(kt == 0),
                                stop=(kt == KT - 1),
                            )
                else:
                    # kt-outer/nb-inner: one stationary load per kt serves 4
                    # matmuls; LDWEIGHTS for kt+1 is pulled ahead by the PE's
                    # reorder window and hides behind kt's matmuls
                    for kt in range(KT):
                        for nb in range(NB):
                            nc.tensor.matmul(
                                psums[nb][:],
                                xt_t[:, kt * P:(kt + 1) * P],
                                wq_sb[:, nb, kt * NBS:(kt + 1) * NBS],
                                start=(kt == 0),
                                stop=(kt == KT - 1),
                            )
                osb = osbp.tile([P, NS], F32, tag="osb", name=f"osb_{j}")
                if j == MT - 1:
                    # pipeline the final drain per n-block so the last store
                    # isn't gated on the full 2048-wide scale+bias pass
                    for nb in range(NB):
                        sl = slice(nb * NBS, (nb + 1) * NBS)
                        nc.vector.tensor_scalar(
                            osb[:, sl], psums[nb][:], scal[:, 0:1], None,
                            mybir.AluOpType.mult,
                        )
                        nc.vector.tensor_tensor(
                            osb[:, sl], osb[:, sl], bias_sb[:, sl],
                            mybir.AluOpType.add,
                        )
                        nc.scalar.dma_start(
                            out=y_out[j * P:(j + 1) * P, sl], in_=osb[:, sl]
                        )
                else:
                    for nb in range(NB):
                        nc.vector.tensor_scalar(
                            osb[:, nb * NBS:(nb + 1) * NBS],
                            psums[nb][:],
                            scal[:, 0:1],
                            None,
                            mybir.AluOpType.mult,
                        )
                    nc.vector.tensor_tensor(
                        osb[:], osb[:], bias_sb[:], mybir.AluOpType.add
                    )
                    nc.scalar.dma_start(out=y_out[j * P:(j + 1) * P, :], in_=osb[:])

    nc.compile()
    return nc


def _compute_gamma(weight: np.ndarray) -> np.float32:
    """Replicate the module's gamma computation bit-exactly (jnp, fp32)."""
    import jax
    import jax.numpy as jnp

    with jax.default_device(jax.devices("cpu")[0]):
        w_f32 = jnp.clip(jnp.asarray(weight, dtype=jnp.float32), -2.0, 2.0)
        gamma = jnp.maximum(jnp.mean(jnp.abs(w_f32)), 1e-4)
        return np.float32(np.asarray(gamma))


def kernel(x: np.ndarray, weight: np.ndarray, bias: np.ndarray) -> np.ndarray:
    global _NC_CACHE, LAST_RESULTS

    x2d = np.asarray(x, dtype=np.float32).reshape(M, K)
    weight = np.asarray(weight, dtype=np.float32)
    bias = np.asarray(bias, dtype=np.float32)

    gamma = _compute_gamma(weight)
    scal = np.full((P, 1), gamma, dtype=np.float32)

    # x: bf16 cast (same RNE rounding the reference path would see from a
    # device-side cast), tiled to [MT, P(k_sub), kt*128+m]
    xt = np.ascontiguousarray(
        x2d.astype(ml_dtypes.bfloat16)
        .reshape(MT, P, KT, P)
        .transpose(0, 3, 2, 1)
        .reshape(MT, P, K)
    )

    # ternary quantization, exact fp32 math as in the reference
    w_f32 = np.clip(weight, -2.0, 2.0)
    w_t = np.clip(np.round(w_f32 / gamma), -1.0, 1.0).astype(ml_dtypes.bfloat16)

    if _NC_CACHE is None:
        _NC_CACHE = _build_nc()
    nc = _NC_CACHE

    in_maps = []
    for i in range(N_CORES):
        wq_shard = np.ascontiguousarray(
            w_t[i * NS:(i + 1) * NS]              # [2048, 4096] ternary bf16
            .reshape(NB, NBS, KT, P)
            .transpose(0, 3, 2, 1)                # [nb, p, kt, n]
            .reshape(NB, P, KT * NBS)
        )
        b_shard = np.ascontiguousarray(
            np.broadcast_to(bias[i * NS:(i + 1) * NS], (P, NS))
        )
        in_maps.append({"xt": xt, "wq": wq_shard, "bias": b_shard, "scal": scal})

    res = run_bass_kernel_spmd(nc, in_maps, list(range(N_CORES)))
    LAST_RESULTS = res

    out = np.concatenate([res.results[i]["out"] for i in range(N_CORES)], axis=1)
    return np.ascontiguousarray(out.reshape(B, S, D_OUT))


# revision 9
# speedup vs baseline: 1.2175x; 1.0078x over previous
"""BitLinear (ternary-quantized linear) Trainium2 kernel.

out = x @ (gamma * ternary(weight)).T + bias, computed tensor-parallel over
8 NeuronCores: weight/bias sharded along out_features, x replicated.

The device program is a pure bf16 matmul streamer: all input preparation
(gamma, ternary quantization of the weight shard, bf16 cast of x, and layout
tiling so every DMA line is long and contiguous) happens on host, where it is
exact fp32 math identical to the reference's. Per core:

  1. DMA the pre-tiled quantized weight shard (4 x 4 MiB, n-block-major, on
     the Scalar queue) and x tiles (pairs of m-tiles, on the Sync queue) into
     SBUF. Each HWDGE queue has ~20us fixed startup to first completion, so
     the first-matmul gate is one 2 MiB xt DMA + one 4 MiB wq chunk in
     parallel.
  2. 8192 bf16 128x128x512 matmuls accumulating fp32 in PSUM. The first
     m-tile pair runs nb-outer interleaved across both m-tiles, so the PE
     consumes each 4 MiB weight chunk over ~14us while the next one arrives
     (~11us) - the weight-load ramp stays PE-bound. Steady state runs
     kt-outer/nb-inner so one LDWEIGHTS (stationary x tile) serves 4 matmuls
     and is hidden by the PE's reorder window.
  3. Drain: psum * gamma on DVE (4 psum banks), + bias, DMA out on the
     Scalar queue; the final m-tile drains per n-block so the last store is
     not gated on a full 2048-wide pass.

gamma = max(mean(|clip(w, -2, 2)|), 1e-4) is computed on host with the same
jnp ops the module uses so the quantization boundary matches bit-exactly.
"""

import numpy as np
import ml_dtypes

import concourse.mybir as mybir
import concourse.tile as tile
from concourse import bacc
from concourse.bass_utils import run_bass_kernel_spmd

P = 128
B, S, D_IN, D_OUT = 4, 2048, 4096, 16384
M = B * S                 # 8192 tokens
K = D_IN                  # 4096 contraction
N_CORES = 8
NS = D_OUT // N_CORES     # 2048 out-features per core
KT = K // P               # 32 k-subtiles
MT = M // P               # 64 m-tiles
MP = MT // 2              # 32 m-tile pairs
NBS = 512                 # psum bank free size (fp32)
NB = NS // NBS            # 4 psum n-blocks

F32 = mybir.dt.float32
BF16 = mybir.dt.bfloat16

_NC_CACHE = None
LAST_RESULTS = None


def _build_nc():
    nc = bacc.Bacc(None, target_bir_lowering=False, debug=False)

    # host-tiled inputs:
    #   xt[jp][p][jj*K + kt*128 + m] = x[(2*jp+jj)*128 + m, kt*128 + p]
    #   wq[nb][p][kt*512 + n]        = ternary_w[nb*512 + n, kt*128 + p]
    xt_in = nc.declare_dram_parameter("xt", [MP, P, 2 * K], BF16, isOutput=False)
    wq_in = nc.declare_dram_parameter("wq", [NB, P, KT * NBS], BF16, isOutput=False)
    b_in = nc.declare_dram_parameter("bias", [P, NS], F32, isOutput=False)
    s_in = nc.declare_dram_parameter("scal", [P, 1], F32, isOutput=False)
    y_out = nc.declare_dram_parameter("out", [M, NS], F32, isOutput=True)

    with tile.TileContext(nc) as tc:
        with (
            tc.tile_pool(name="const", bufs=1) as constp,
            tc.tile_pool(name="xt", bufs=2) as xtp,
            tc.tile_pool(name="osb", bufs=3) as osbp,
            tc.tile_pool(name="psum", bufs=8, space="PSUM") as psump,
        ):
            wq_sb = constp.tile([P, NB, KT * NBS], BF16)
            scal = constp.tile([P, 1], F32)
            bias_sb = constp.tile([P, NS], F32)

            def drain(j, psums, pipelined):
                osb = osbp.tile([P, NS], F32, tag="osb", name=f"osb_{j}")
                if pipelined:
                    for nb in range(NB):
                        sl = slice(nb * NBS, (nb + 1) * NBS)
                        nc.vector.tensor_scalar(
                            osb[:, sl], psums[nb][:], scal[:, 0:1], None,
                            mybir.AluOpType.mult,
                        )
                        nc.vector.tensor_tensor(
                            osb[:, sl], osb[:, sl], bias_sb[:, sl],
                            mybir.AluOpType.add,
                        )
                        nc.scalar.dma_start(
                            out=y_out[j * P:(j + 1) * P, sl], in_=osb[:, sl]
                        )
                else:
                    for nb in range(NB):
                        nc.vector.tensor_scalar(
                            osb[:, nb * NBS:(nb + 1) * NBS],
                            psums[nb][:],
                            scal[:, 0:1],
                            None,
                            mybir.AluOpType.mult,
                        )
                    nc.vector.tensor_tensor(
                        osb[:], osb[:], bias_sb[:], mybir.AluOpType.add
                    )
                    nc.scalar.dma_start(
                        out=y_out[j * P:(j + 1) * P, :], in_=osb[:]
                    )

            for jp in range(MP):
                xt_t = xtp.tile([P, 2 * K], BF16, tag="xt", name=f"xt_{jp}")
                nc.sync.dma_start(out=xt_t[:], in_=xt_in[jp])
                if jp == 0:
                    # weight chunks + bias on the Scalar queue; scal behind
                    # xt pair 0 on Sync. (SWDGE/gpsimd is avoided: its
                    # software descriptor startup stalls DMA ~30us at start.)
                    nc.sync.dma_start(out=scal[:], in_=s_in[:])
                    for nb in range(NB):
                        nc.scalar.dma_start(out=wq_sb[:, nb, :], in_=wq_in[nb])
                    nc.scalar.dma_start(out=bias_sb[:], in_=b_in[:])
                psums = [
                    [
                        psump.tile([P, NBS], F32, tag="ps", name=f"ps_{jp}_{jj}_{nb}")
                        for nb in range(NB)
                    ]
                    for jj in range(2)
                ]
                if jp == 0:
                    # ramp: nb-outer, interleaved across the two m-tiles of
                    # the pair - each weight chunk feeds ~14us of matmuls
                    # while the next chunk (~11us) arrives
                    for nb in range(NB):
                        for jj in range(2):
                            for kt in range(KT):
                                nc.tensor.matmul(
                                    psums[jj][nb][:],
                                    xt_t[:, jj * K + kt * P:jj * K + (kt + 1) * P],
                                    wq_sb[:, nb, kt * NBS:(kt + 1) * NBS],
                                    start=(kt == 0),
                                    stop=(kt == KT - 1),
                                )
                else:
                    for jj in range(2):
                        for kt in range(KT):
                            for nb in range(NB):
                                nc.tensor.matmul(
                                    psums[jj][nb][:],
                                    xt_t[:, jj * K + kt * P:jj * K + (kt + 1) * P],
                                    wq_sb[:, nb, kt * NBS:(kt + 1) * NBS],
                                    start=(kt == 0),
                                    stop=(kt == KT - 1),
                                )
                for jj in range(2):
                    j = 2 * jp + jj
                    drain(j, psums[jj], pipelined=(j == MT - 1))

    nc.compile()
    return nc


def _compute_gamma(weight: np.ndarray) -> np.float32:
    """Replicate the module's gamma computation bit-exactly (jnp, fp32)."""
    import jax
    import jax.numpy as jnp

    with jax.default_device(jax.devices("cpu")[0]):
        w_f32 = jnp.clip(jnp.asarray(weight, dtype=jnp.float32), -2.0, 2.0)
        gamma = jnp.maximum(jnp.mean(jnp.abs(w_f32)), 1e-4)
        return np.float32(np.asarray(gamma))


def kernel(x: np.ndarray, weight: np.ndarray, bias: np.ndarray) -> np.ndarray:
    global _NC_CACHE, LAST_RESULTS

    x2d = np.asarray(x, dtype=np.float32).reshape(M, K)
    weight = np.asarray(weight, dtype=np.float32)
    bias = np.asarray(bias, dtype=np.float32)

    gamma = _compute_gamma(weight)
    scal = np.full((P, 1), gamma, dtype=np.float32)

    # x: bf16 cast (RNE, same rounding a device-side cast would apply),
    # tiled to [jp, p(k_sub), (jj, kt, m)]
    xt = np.ascontiguousarray(
        x2d.astype(ml_dtypes.bfloat16)
        .reshape(MP, 2, P, KT, P)         # [jp, jj, m, kt, p]
        .transpose(0, 4, 1, 3, 2)         # [jp, p, jj, kt, m]
        .reshape(MP, P, 2 * K)
    )

    # ternary quantization, exact fp32 math as in the reference
    w_f32 = np.clip(weight, -2.0, 2.0)
    w_t = np.clip(np.round(w_f32 / gamma), -1.0, 1.0).astype(ml_dtypes.bfloat16)

    if _NC_CACHE is None:
        _NC_CACHE = _build_nc()
    nc = _NC_CACHE

    in_maps = []
    for i in range(N_CORES):
        wq_shard = np.ascontiguousarray(
            w_t[i * NS:(i + 1) * NS]              # [2048, 4096] ternary bf16
            .reshape(NB, NBS, KT, P)
            .transpose(0, 3, 2, 1)                # [nb, p, kt, n]
            .reshape(NB, P, KT * NBS)
        )
        b_shard = np.ascontiguousarray(
            np.broadcast_to(bias[i * NS:(i + 1) * NS], (P, NS))
        )
        in_maps.append({"xt": xt, "wq": wq_shard, "bias": b_shard, "scal": scal})

    res = run_bass_kernel_spmd(nc, in_maps, list(range(N_CORES)))
    LAST_RESULTS = res

    out = np.concatenate([res.results[i]["out"] for i in range(N_CORES)], axis=1)
    return np.ascontiguousarray(out.reshape(B, S, D_OUT))
